# revision 39
# baseline (speedup 1.0000x reference)
"""Sliding-window MQA attention block on 8 Trainium2 NeuronCores.

Sharding: sequence-parallel. 8 cores = 2 batches x 4 query-chunks of 512
tokens. Each core loads its 512 query tokens plus a 256-token K/V halo
(768 KV tokens total, zero-padded in front for chunk 0), computes the
Q/K/V projections, windowed attention for all 16 heads, and the final
projection locally. No collectives; the host concatenates chunk outputs.

The weight matrices, bias, and the (core-independent) sliding-window
mask are embedded in the NEFF as Const tensors (loaded to HBM once at
model-load time), so the only per-execution inputs are the core's
768-token activation slice (bf16, 1.5MB) and a tiny key-validity
column (1.5KB) that zeroes the softmax-denominator contributions of
chunk-0's zero-padded halo keys (their V rows are already zero). This
minimizes per-dispatch input staging traffic, which dominates the
measured execution time on this stack.

Device algorithm (per core), logits kept in [t, s] orientation:
  qT[1024, 512]  = WqT.T @ xqT        (per 128-row blocks)
  ktd[128, 768]  = K^T duplicated into both partition halves (MQA shared)
  v_aug[768, 65] = V with an all-ones column (gives softmax denominators)
  per head h, per 128-query block tb (s-window = 384 = 128 + 256 halo):
    logits[128, 384] = qh_tb.T @ kT[:, window]
    probs = exp(0.125 * logits) * band   (band = 0/1 sliding-window mask)
    probsT pieces via PE transpose; out[t, 65] = sum_sb probsT_sb.T @ v_aug
    attn[t, 64h:64h+64] = out[:, :64] * (1 / out[:, 64])
  attnT via PE transpose; final[512, 1024] = attnT.T @ WfT + bias
"""

import os
import sys

import numpy as np

for _p in ("/opt/trn_rl_repo",):
    if _p not in sys.path and os.path.isdir(_p):
        sys.path.insert(0, _p)

import ml_dtypes

import concourse.mybir as mybir
import concourse.tile as tile
from concourse import bacc
from concourse.bass_utils import run_bass_kernel_spmd
from concourse.masks import make_identity

WIDTH = 1024
H = 16
HD = 64
WIN = 256
T = 512          # query tokens per core
KV = 768         # kv tokens per core (256 halo + 512)
NKB = WIDTH // 128
NTB = T // 128
NSB = KV // 128
WINW = 384       # s-window per 128-query block
F32 = mybir.dt.float32
BF16 = mybir.dt.bfloat16

NPDT = ml_dtypes.bfloat16

# tile-pool buffer counts (overlap depth); sim-tuned
PSQ_BUFS = 3
PSV_BUFS = 3
PSL_BUFS = 2
PST_BUFS = 2
PSO_BUFS = 2
AWORK_BUFS = 3
PSAT_BUFS = 2
PSF_BUFS = 4
FIN_BUFS = 3

# engine assignment for data-movement / elementwise ops ("vector" = DVE,
# "gpsimd" = Pool, "scalar" = Activation). GPSIMD cannot read PSUM on
# real hardware, so PSUM->SBUF copies go to DVE or Activation and only
# SBUF->SBUF work (the band multiply) goes to Pool; sim-tuned.


def _copy(nc, engine, out, in_):
    if engine == "scalar":
        nc.scalar.copy(out, in_)
    else:
        getattr(nc, engine).tensor_copy(out, in_)


PTS_ENGINE = "vector"      # probsT PSUM->SBUF copies (64x [128,384])
ATTNT_ENGINE = "vector"    # attnT PSUM->SBUF copies (8x [128,512])
PROBSM_ENGINES = ("gpsimd", "gpsimd")  # band multiply, by head parity
NORM_ENGINES = ("vector", "vector")    # attn normalize, by head parity
DT = BF16


def build_kernel(Wq=None, Wk=None, Wv=None, Wf=None, bf=None, reps=1,
                 packed_x=False):
    """Build the per-core Bass module. Weights are NEFF-embedded consts."""
    if Wq is None:
        Wq = np.zeros((WIDTH, WIDTH), np.float32)
    if Wk is None:
        Wk = np.zeros((HD, WIDTH), np.float32)
    if Wv is None:
        Wv = np.zeros((HD, WIDTH), np.float32)
    if Wf is None:
        Wf = np.zeros((WIDTH, WIDTH), np.float32)
    if bf is None:
        bf = np.zeros((WIDTH,), np.float32)

    nc = bacc.Bacc(None, target_bir_lowering=False)

    if packed_x:
        xkvT_d = nc.dram_tensor("xkvT", [128, NKB, KV], DT, kind="ExternalInput")
    else:
        xkvT_d = nc.dram_tensor("xkvT", [WIDTH, KV], DT, kind="ExternalInput")
    vcol_d = nc.dram_tensor("vcol", [128, NSB], DT, kind="ExternalInput")
    out_d = nc.dram_tensor("out", [T, WIDTH], F32, kind="ExternalOutput")

    wqT_d = nc.inline_tensor(np.ascontiguousarray(Wq.T).astype(NPDT), name="wqT")
    wkT_d = nc.inline_tensor(np.ascontiguousarray(Wk.T).astype(NPDT), name="wkT")
    wvT_d = nc.inline_tensor(np.ascontiguousarray(Wv.T).astype(NPDT), name="wvT")
    wfT_d = nc.inline_tensor(np.ascontiguousarray(Wf.T).astype(NPDT), name="wfT")
    bias_d = nc.inline_tensor(
        np.ascontiguousarray(np.broadcast_to(bf.astype(np.float32), (128, WIDTH))),
        name="biasb",
    )
    # Sliding-window mask, core-independent: the chunk-0 "key position < 0"
    # padding is handled by vcol zeroing those keys' softmax-denominator
    # contributions (their V rows are already zero since x is zero-padded).
    p_ = np.arange(128)[:, None]
    f_ = np.arange(WINW)[None, :]
    base_band = ((f_ - p_ >= 0) & (f_ - p_ <= WIN)).astype(np.float32)
    band_d = nc.inline_tensor(
        np.ascontiguousarray(
            np.broadcast_to(base_band[:, None, :], (128, NTB, WINW))
        ).astype(NPDT),
        name="bandc",
    )

    with tile.TileContext(nc) as tc:
        with tc.tile_pool(name="persist", bufs=1) as pp:
            # ---- load inputs ----
            if packed_x:
                xkv_big = pp.tile([128, NKB, KV], DT, tag="xkvall", name="xkvall")
                nc.sync.dma_start(xkv_big[:], xkvT_d[:, :, :])
                xkv_t = [xkv_big[:, i, :] for i in range(NKB)]
            else:
                xkv_t = [
                    pp.tile([128, KV], DT, tag=f"xkv{i}", name=f"xkv{i}")
                    for i in range(NKB)
                ]
            # Interleave (wk_i, xkv_i) so the K-projection's kb=0 matmul can
            # issue after just two DMAs instead of waiting for whole tensors.
            # wf goes last (only needed by the final projection).
            wk_t = []
            wv_t = []
            for i in range(NKB):
                t_ = pp.tile([128, HD], DT, tag=f"wk{i}", name=f"wk{i}")
                nc.sync.dma_start(t_[:], wkT_d[128 * i : 128 * (i + 1), :])
                wk_t.append(t_)
                if not packed_x:
                    nc.sync.dma_start(
                        xkv_t[i][:], xkvT_d[128 * i : 128 * (i + 1), :]
                    )
            for i in range(NKB):
                t_ = pp.tile([128, HD], DT, tag=f"wv{i}", name=f"wv{i}")
                nc.sync.dma_start(t_[:], wvT_d[128 * i : 128 * (i + 1), :])
                wv_t.append(t_)
            wq_t = []
            for i in range(NKB):
                t_ = pp.tile([128, WIDTH], DT, tag=f"wq{i}", name=f"wq{i}")
                nc.sync.dma_start(t_[:], wqT_d[128 * i : 128 * (i + 1), :])
                wq_t.append(t_)
            wf_t = []
            for i in range(NKB):
                t_ = pp.tile([128, WIDTH], DT, tag=f"wf{i}", name=f"wf{i}")
                nc.sync.dma_start(t_[:], wfT_d[128 * i : 128 * (i + 1), :])
                wf_t.append(t_)
            band_t = pp.tile([128, NTB, WINW], DT, tag="band")
            nc.sync.dma_start(band_t[:], band_d[:, :, :])
            vcol_t = pp.tile([128, NSB], DT, tag="vcol")
            nc.sync.dma_start(vcol_t[:], vcol_d[:, :])
            bias_t = pp.tile([128, WIDTH], F32, tag="bias")
            nc.sync.dma_start(bias_t[:], bias_d[:, :])

            ident = pp.tile([128, 128], DT, tag="ident")
            make_identity(nc, ident[:])

            for _rep in range(reps):
                _build_body(nc, tc, pp, xkv_t, wq_t, wf_t, wk_t, wv_t,
                            band_t, vcol_t, bias_t, ident, out_d)

    return nc


def _build_body(nc, tc, pp, xkv_t, wq_t, wf_t, wk_t, wv_t, band_t, vcol_t,
                bias_t, ident, out_d):
    if True:
        if True:
            # ---- persistent intermediates ----
            qT_t = [pp.tile([128, T], DT, tag=f"qT{i}", name=f"qT{i}") for i in range(NKB)]
            ktd = pp.tile([128, KV], DT, tag="ktd")
            vaug = [pp.tile([128, HD + 1], DT, tag=f"vaug{i}", name=f"vaug{i}") for i in range(NSB)]
            attn_t = [pp.tile([128, WIDTH], DT, tag=f"attn{i}", name=f"attn{i}") for i in range(NTB)]
            attnT_t = [pp.tile([128, T], DT, tag=f"attnT{i}", name=f"attnT{i}") for i in range(NKB)]

            # ---- phase 1: projections ----
            with (
                tc.tile_pool(name="psq", bufs=PSQ_BUFS, space="PSUM") as psq_pool,
                tc.tile_pool(name="psk", bufs=1, space="PSUM") as psk_pool,
                tc.tile_pool(name="psv", bufs=PSV_BUFS, space="PSUM") as psv_pool,
            ):
                for mb in range(NKB):
                    pq = psq_pool.tile([128, T], F32, tag="pq")
                    for kb in range(NKB):
                        nc.tensor.matmul(
                            pq[:],
                            lhsT=wq_t[kb][:, 128 * mb : 128 * (mb + 1)],
                            rhs=xkv_t[kb][:, WIN : WIN + T],
                            start=(kb == 0),
                            stop=(kb == NKB - 1),
                        )
                    nc.vector.tensor_copy(qT_t[mb][:], pq[:])

                pk = psk_pool.tile([128, KV], F32, tag="pk")
                for seg0, segw in ((0, 512), (512, 256)):
                    for kb in range(NKB):
                        nc.tensor.matmul(
                            pk[0:64, seg0 : seg0 + segw],
                            lhsT=wk_t[kb][:],
                            rhs=xkv_t[kb][:, seg0 : seg0 + segw],
                            start=(kb == 0),
                            stop=(kb == NKB - 1),
                        )
                nc.vector.tensor_copy(ktd[0:64, :], pk[0:64, :])
                # duplicate K^T into the upper partition half (MQA: both
                # head-halves share the same K); SBUF->SBUF DMA shifts
                # partitions, which no compute engine can.
                nc.sync.dma_start(ktd[64:128, :], ktd[0:64, :])

                for sb in range(NSB):
                    pv = psv_pool.tile([128, HD], F32, tag="pv")
                    for kb in range(NKB):
                        nc.tensor.matmul(
                            pv[:],
                            lhsT=xkv_t[kb][:, 128 * sb : 128 * (sb + 1)],
                            rhs=wv_t[kb][:],
                            start=(kb == 0),
                            stop=(kb == NKB - 1),
                        )
                    nc.scalar.copy(vaug[sb][:, 0:HD], pv[:])
                    nc.vector.tensor_copy(
                        vaug[sb][:, HD : HD + 1], vcol_t[:, sb : sb + 1]
                    )

            # ---- phase 2: attention ----
            with (
                tc.tile_pool(name="psl", bufs=PSL_BUFS, space="PSUM") as psl_pool,
                tc.tile_pool(name="pst", bufs=PST_BUFS, space="PSUM") as pst_pool,
                tc.tile_pool(name="pso", bufs=PSO_BUFS, space="PSUM") as pso_pool,
                tc.tile_pool(name="awork", bufs=AWORK_BUFS) as awork,
            ):
                def front(h):
                    """logits -> exp -> band mask for head h."""
                    mb, half = divmod(h, 2)
                    hb = 64 * half
                    qh = qT_t[mb]
                    probs = awork.tile([128, NTB, WINW], DT, tag="probs")
                    for pair in range(2):
                        pl = psl_pool.tile([128, 2, 512], F32, tag="pl")
                        for u in range(2):
                            tb = 2 * pair + u
                            nc.tensor.matmul(
                                pl[:, u, 0:WINW],
                                lhsT=qh[hb : hb + 64, 128 * tb : 128 * (tb + 1)],
                                rhs=ktd[hb : hb + 64, 128 * tb : 128 * tb + WINW],
                                start=True,
                                stop=True,
                            )
                        nc.scalar.activation(
                            out=probs[:, 2 * pair : 2 * pair + 2, :],
                            in_=pl[:, :, 0:WINW],
                            func=mybir.ActivationFunctionType.Exp,
                            scale=0.125,
                        )
                    probsm = awork.tile([128, NTB, WINW], DT, tag="probsm")
                    getattr(nc, PROBSM_ENGINES[h % 2]).tensor_mul(
                        probsm[:], probs[:], band_t[:]
                    )
                    return probsm

                def back(h, probsm):
                    """transpose -> PV -> normalize for head h."""
                    po = pso_pool.tile([128, NTB, 128], F32, tag="po")
                    for tb in range(NTB):
                        pt = pst_pool.tile([128, WINW], DT, tag="pt")
                        for k3 in range(3):
                            nc.tensor.transpose(
                                pt[:, 128 * k3 : 128 * (k3 + 1)],
                                probsm[:, tb, 128 * k3 : 128 * (k3 + 1)],
                                ident[:],
                            )
                        pT_sb = awork.tile([128, WINW], DT, tag="pTs")
                        _copy(nc, PTS_ENGINE, pT_sb[:], pt[:])
                        for k3 in range(3):
                            nc.tensor.matmul(
                                po[:, tb, 0 : HD + 1],
                                lhsT=pT_sb[:, 128 * k3 : 128 * (k3 + 1)],
                                rhs=vaug[tb + k3][:],
                                start=(k3 == 0),
                                stop=(k3 == 2),
                            )
                    recip = awork.tile([128, NTB, 1], F32, tag="recip")
                    nc.vector.reciprocal(recip[:], po[:, :, HD : HD + 1])
                    for tb in range(NTB):
                        getattr(nc, NORM_ENGINES[h % 2]).tensor_scalar_mul(
                            attn_t[tb][:, 64 * h : 64 * (h + 1)],
                            po[:, tb, 0:HD],
                            recip[:, tb, :],
                        )

                # software pipeline: head h+1's logits are emitted before
                # head h's transpose/PV chain, so the in-order PE stream
                # never stalls waiting on Act/DVE of the current head.
                pm = front(0)
                for h in range(H):
                    pm_next = front(h + 1) if h + 1 < H else None
                    back(h, pm)
                    pm = pm_next

            # attn -> attnT for the final projection
            with (
                tc.tile_pool(name="psat", bufs=PSAT_BUFS, space="PSUM") as psat_pool,
            ):
                for wb in range(NKB):
                    pat = psat_pool.tile([128, NTB, 128], DT, tag="pat")
                    for tb in range(NTB):
                        nc.tensor.transpose(
                            pat[:, tb, :],
                            attn_t[tb][:, 128 * wb : 128 * (wb + 1)],
                            ident[:],
                        )
                    _copy(nc, ATTNT_ENGINE, attnT_t[wb][:], pat[:])

            # ---- phase 3: final projection + bias ----
            with (
                tc.tile_pool(name="psf", bufs=PSF_BUFS, space="PSUM") as psf_pool,
                tc.tile_pool(name="fin", bufs=FIN_BUFS) as fin_pool,
            ):
                for tb in range(NTB):
                    for nh in range(2):
                        pf = psf_pool.tile([128, 512], F32, tag="pf")
                        for wb in range(NKB):
                            nc.tensor.matmul(
                                pf[:],
                                lhsT=attnT_t[wb][:, 128 * tb : 128 * (tb + 1)],
                                rhs=wf_t[wb][:, 512 * nh : 512 * (nh + 1)],
                                start=(wb == 0),
                                stop=(wb == NKB - 1),
                            )
                        fo = fin_pool.tile([128, 512], F32, tag="fo")
                        nc.vector.tensor_add(
                            fo[:], pf[:], bias_t[:, 512 * nh : 512 * (nh + 1)]
                        )
                        nc.sync.dma_start(
                            out_d[128 * tb : 128 * (tb + 1), 512 * nh : 512 * (nh + 1)],
                            fo[:],
                        )


def _prep_core_inputs(x, core, packed=False):
    """Per-core, per-execution inputs: activation slice + key-validity column."""
    bi, ch = divmod(core, 4)
    qs = T * ch
    ks = qs - WIN
    xkvT = np.zeros((WIDTH, KV), np.float32)
    lo = max(ks, 0)
    xkvT[:, lo - ks :] = x[bi, lo : qs + T, :].T

    # vcol[p, sb] = 1 if key position ks + 128*sb + p is a real token (>= 0)
    j = np.arange(128)[:, None] + 128 * np.arange(NSB)[None, :]
    vcol = (ks + j >= 0).astype(np.float32)

    if packed:
        xkvT = xkvT.reshape(NKB, 128, KV).transpose(1, 0, 2)
    return {
        "xkvT": np.ascontiguousarray(xkvT).astype(NPDT),
        "vcol": vcol.astype(NPDT),
    }


_RUN_KW = {}  # test.py can inject trace=True etc.
_LAST_RESULT = [None]


def kernel(x, segment_pos, Wq, Wk, Wv, Wf, bf):
    x = np.asarray(x, np.float32)
    Wq = np.asarray(Wq, np.float32)
    Wk = np.asarray(Wk, np.float32)
    Wv = np.asarray(Wv, np.float32)
    Wf = np.asarray(Wf, np.float32)
    bf = np.asarray(bf, np.float32)

    nc = build_kernel(Wq, Wk, Wv, Wf, bf)
    nc.finalize()
    in_maps = [_prep_core_inputs(x, c) for c in range(8)]
    res = run_bass_kernel_spmd(nc, in_maps, core_ids=list(range(8)), **_RUN_KW)
    _LAST_RESULT[0] = res

    b, t = x.shape[0], x.shape[1]
    out = np.empty((b, t, WIDTH), np.float32)
    for c in range(8):
        bi, ch = divmod(c, 4)
        out[bi, T * ch : T * (ch + 1)] = res.results[c]["out"]
    return out
